# revision 27
# baseline (speedup 1.0000x reference)
"""Binarized 3x3 conv + batchnorm(train) + sign, on 8 TRN2 NeuronCores.

Math: out = sign((y - mean)/sqrt(var+eps)) where y = conv(x, sign(w)) + sign(b)
and mean/var are per-channel batch stats.  Since sqrt(var+eps) > 0, the output
is exactly sign(y - mean_c): variance never needs to be computed.  The +-1
channel bias cancels in sign(y - mean), so it is dropped entirely.

Strategy (data-parallel over batch, 4 images/core):
 - implicit GEMM, all matmuls in fp8-e4m3 DoubleRow perf mode (0.5 PE
   cycles/output-row, 2x the fp16 rate; both 128-ci blocks folded into one
   matmul via the DR pair dim).
 - fp32-quality precision from a 3-component split:
       x ~= c1 + c2/64 + c3/64',  c1 = e4m3(x), c2 = e4m3(64*(x-c1)),
       c3 = e4m3(64*(x - c1 - c2/64))
   The /64 scales are folded into the conv weights: comp-1 weights are +-1,
   comp-2/3 weights are +-2^-6 (exact in e4m3, and +-1-weight products are
   exact sign flips).  All 27 matmuls per output tile (3 comps x 9 taps)
   accumulate into ONE fp32 PSUM bank -> a single drain per tile.
   Measured on the reference inputs: 138/23.9M sign flips (rel err 4.8e-3).
 - per-tile drain on ScalarE copies PSUM->SBUF and harvests per-channel sums
   for free (accum_out), leaving VectorE clear for the binarize pass.
 - one tiny AllReduce (128x2 fp32) across the 8 cores for the global mean.
 - pass 2: binarize split over VectorE (is_ge -> {0,1}) and ScalarE
   (Sign -> {-1,0,1}) per BIN_ENG, DMA out as 1 byte/elem; host maps each
   block back to +-1 fp32.
"""

import sys

if "/opt/trn_rl_repo" not in sys.path:
    sys.path.insert(0, "/opt/trn_rl_repo")

import numpy as np
import ml_dtypes

N_CORES = 8
N_PER_CORE = 4          # images per core
CI = 256                # in channels
CO = 256                # out channels
H = W = 56
OH = OW = 54
HWF = H * W             # 3136
HWPAD = HWF + 16        # fp8 image length in SBUF; pair-dim stride 16B-aligned
NPIX = OH * OW          # 2916
RT = 6                  # row tiles per image (9 rows each)
RROWS = 9
FREE = RROWS * W        # 504 raw row span
TFREE = RROWS * OW      # 486 valid outputs per tile
N_TOT = N_CORES * N_PER_CORE
MEAN_SCALE = 1.0 / (N_TOT * NPIX)
C_SCALE = 64.0          # residual components stored at 64x, weights at 1/64
NT = N_PER_CORE * 2 * RT  # 48 tiles per core

# engine per phase-3 binarize block (cb*4+n): v=DVE is_ge {0,1},
# a=ScalarE Sign {-1,0,1}; all stored as fp8 bytes
BIN_ENG = ["v", "a", "v", "a", "v", "a", "v", "v"]

FP8 = ml_dtypes.float8_e4m3


def build(nc, n_cores=N_CORES):
    """Emit the SPMD program into a bacc.Bacc instance."""
    import concourse.mybir as mybir
    from concourse import tile

    f32 = mybir.dt.float32
    fp8 = mybir.dt.float8e4
    ACT = mybir.ActivationFunctionType
    DR = mybir.MatmulPerfMode.DoubleRow

    x_d = [
        nc.dram_tensor(f"x{c}", [N_PER_CORE, 128, 2, HWPAD], fp8, kind="ExternalInput")
        for c in range(3)
    ]
    w1_d = nc.dram_tensor("w1", [128, 2, 9, 2, 128], fp8, kind="ExternalInput")
    ws_d = nc.dram_tensor("ws", [128, 2, 9, 2, 128], fp8, kind="ExternalInput")
    y_d = nc.dram_tensor("y", [N_PER_CORE, 2, 128, NPIX], mybir.dt.uint8, kind="ExternalOutput")

    with tile.TileContext(nc) as tc:
        with (
            tc.tile_pool(name="wpool", bufs=1) as wpool,
            tc.tile_pool(name="xpool", bufs=2) as xpool,
            tc.tile_pool(name="ypool", bufs=1) as ypool,
            tc.tile_pool(name="spool", bufs=1) as spool,
            tc.tile_pool(name="opool", bufs=8) as opool,
            tc.tile_pool(name="pspool", bufs=8, space="PSUM") as pspool,
            tc.tile_pool(name="drampool", bufs=2, space="DRAM") as drampool,
        ):
            w1_sb = wpool.tile([128, 2, 9, 2, 128], fp8, tag="w1")
            ws_sb = wpool.tile([128, 2, 9, 2, 128], fp8, tag="ws")
            y_sb = ypool.tile([128, NT * TFREE], f32)
            sums = spool.tile([128, NT], f32, tag="sums")

            # ---------------- phase 1: conv + drain (+sums) ------------------
            # All DMA transfers serialize on the HWDGE device, so the startup
            # transfers are ordered by first use: w1[cb0] + head of x0 (tile
            # 0's comp-1 matmuls), then ws[cb0]+x1 head, x2 head, tails, cb1
            # weights.
            HD = 30 * W  # 1680: covers rt 0-2 matmul reads (rows 0..29)
            for n in range(N_PER_CORE):
                xc = [
                    xpool.tile([128, 2, HWPAD], fp8, tag=f"x{c}", name=f"x{c}")
                    for c in range(3)
                ]
                if n == 0:
                    nc.sync.dma_start(w1_sb[:, 0], w1_d[:, 0])
                    nc.scalar.dma_start(xc[0][:, :, 0:HD], x_d[0][n][:, :, 0:HD])
                    nc.sync.dma_start(ws_sb[:, 0], ws_d[:, 0])
                    nc.scalar.dma_start(xc[1][:, :, 0:HD], x_d[1][n][:, :, 0:HD])
                    nc.sync.dma_start(xc[0][:, :, HD:], x_d[0][n][:, :, HD:])
                    nc.scalar.dma_start(xc[2][:, :, 0:HD], x_d[2][n][:, :, 0:HD])
                    nc.sync.dma_start(xc[1][:, :, HD:], x_d[1][n][:, :, HD:])
                    nc.scalar.dma_start(xc[2][:, :, HD:], x_d[2][n][:, :, HD:])
                    nc.sync.dma_start(w1_sb[:, 1], w1_d[:, 1])
                    nc.scalar.dma_start(ws_sb[:, 1], ws_d[:, 1])
                else:
                    nc.scalar.dma_start(xc[0][:], x_d[0][n])
                    nc.gpsimd.dma_start(xc[1][:], x_d[1][n])
                    nc.gpsimd.dma_start(xc[2][:], x_d[2][n])

                for cb in range(2):
                    for rt in range(RT):
                        ps = pspool.tile([128, TFREE], f32, tag="ps")
                        for c in range(3):
                            w_sb = w1_sb if c == 0 else ws_sb
                            for s in range(9):
                                kh, kw = divmod(s, 3)
                                off = (rt * RROWS + kh) * W + kw
                                # 4D rhs view drops the 2 wrap cols per row:
                                # 486-wide DR output (0.5 cyc/row on 486
                                # instead of 504)
                                rhs = xc[c][:, :, off : off + FREE].rearrange(
                                    "p b (r c) -> p b r c", c=W
                                )[:, :, :, 0:OW]
                                nc.tensor.matmul(
                                    ps[:],
                                    w_sb[:, cb, s],
                                    rhs,
                                    start=(c == 0 and s == 0),
                                    stop=(c == 2 and s == 8),
                                    perf_mode=DR,
                                )
                        t = (cb * N_PER_CORE + n) * RT + rt
                        ps_v = ps[:]
                        osl = y_sb[:, t * TFREE : (t + 1) * TFREE]
                        nc.scalar.activation(
                            osl, ps_v, ACT.Copy, accum_out=sums[:, t : t + 1]
                        )

            # ---------------- phase 2: global mean via AllReduce ------------
            sums2 = spool.tile([128, 2], f32, tag="sums2")
            # tile index t = (cb*N + n)*RT + rt, so cb is outermost:
            # one X-axis reduce over the 24 per-cb columns.
            nc.vector.tensor_reduce(
                sums2[:],
                sums[:].rearrange("p (c m) -> p c m", c=2),
                axis=mybir.AxisListType.X,
                op=mybir.AluOpType.add,
            )
            neg_mean = spool.tile([128, 2], f32, tag="negmean")
            if n_cores > 1:
                cc_in = drampool.tile([128, 2], f32)
                cc_out = drampool.tile([128, 2], f32)
                nc.sync.dma_start(cc_in[:], sums2[:])
                nc.gpsimd.collective_compute(
                    "AllReduce",
                    mybir.AluOpType.add,
                    replica_groups=[list(range(n_cores))],
                    ins=[cc_in.opt()],
                    outs=[cc_out.opt()],
                )
                sums_g = spool.tile([128, 2], f32, tag="sumsg")
                nc.sync.dma_start(sums_g[:], cc_out[:])
                nc.scalar.mul(neg_mean[:], sums_g[:], -MEAN_SCALE)
            else:
                # single-core timing variant (TimelineSim can't model
                # collectives): mean is just this core's sums
                nc.scalar.mul(neg_mean[:], sums2[:], -MEAN_SCALE)

            # ---------------- phase 3: binarize + store ---------------------
            # spread the 8 blocks over DVE (is_ge -> {0,1}), ScalarE
            # (Sign -> {-1,0,1}) and GpSimd (is_ge) per BIN_ENG; all 1B fp8
            # out.  Host maps back to +-1 fp32 per block encoding.
            # out-DMAs issue from SP/GpSimd so their issue cost never
            # interleaves with the ScalarE Sign ops
            for cb in range(2):
                for n in range(N_PER_CORE):
                    b = cb * N_PER_CORE + n
                    t0 = b * RT
                    eng = BIN_ENG[b]
                    bin_t = opool.tile([128, RT * TFREE], fp8, tag="bin")
                    y_v = y_sb[:, t0 * TFREE : (t0 + RT) * TFREE]
                    nm = neg_mean[:, cb : cb + 1]
                    if eng == "a":
                        nc.scalar.activation(bin_t[:], y_v, ACT.Sign, bias=nm)
                    else:
                        nc.vector.tensor_scalar(
                            bin_t[:],
                            y_v,
                            nm,
                            0.0,
                            mybir.AluOpType.add,
                            mybir.AluOpType.is_ge,
                        )
                    dma_e = nc.sync if b < 4 else nc.gpsimd
                    dma_e.dma_start(y_d[n, cb], bin_t[:].bitcast(mybir.dt.uint8))

    nc.compile()
    return nc


def prep_inputs(x, weight, bias):
    """Host-side shard + layout prep. Returns list of 8 per-core input maps."""
    assert x.shape == (N_TOT, CI, H, W) and x.dtype == np.float32

    # x -> [core, n, ci_f(p), ci_b, hw]; 3-component e4m3 split
    xs = np.ascontiguousarray(
        x.reshape(N_CORES, N_PER_CORE, 2, 128, HWF).transpose(0, 1, 3, 2, 4)
    )
    c1 = xs.astype(FP8)
    r1 = xs - c1.astype(np.float32)
    c2 = (r1 * np.float32(C_SCALE)).astype(FP8)
    r2 = r1 - c2.astype(np.float32) * np.float32(1.0 / C_SCALE)
    c3 = (r2 * np.float32(C_SCALE)).astype(FP8)
    pad = ((0, 0),) * 4 + ((0, HWPAD - HWF),)
    c1 = np.pad(c1, pad)
    c2 = np.pad(c2, pad)
    c3 = np.pad(c3, pad)

    wb = np.where(weight >= 0, np.float32(1.0), np.float32(-1.0))
    # [co_b, co_f, ci_b, ci_f, kh, kw] -> [ci_f(p), co_b, (kh kw), ci_b, co_f]
    w6 = wb.reshape(2, 128, 2, 128, 3, 3)
    wt = np.ascontiguousarray(w6.transpose(3, 0, 4, 5, 2, 1)).reshape(
        128, 2, 9, 2, 128
    )
    w1 = wt.astype(FP8)
    ws = (wt * np.float32(1.0 / C_SCALE)).astype(FP8)  # +-2^-6, exact
    return [
        {
            "x0": c1[c],
            "x1": c2[c],
            "x2": c3[c],
            "w1": w1,
            "ws": ws,
        }
        for c in range(N_CORES)
    ]


def gather(results):
    """[{y: [4,2,128,2916] fp8}] * 8 -> (32, 256, 54, 54) fp32 +-1.

    DVE/GpSimd blocks hold {0,1} (is_ge), ScalarE blocks hold {-1,0,1}
    (Sign); see BIN_ENG."""
    ys = np.stack([np.asarray(r["y"]).view(FP8) for r in results]).astype(np.float32)
    out = np.empty_like(ys)
    for b, eng in enumerate(BIN_ENG):
        cb, n = divmod(b, N_PER_CORE)
        v = ys[:, n, cb]
        if eng == "a":
            out[:, n, cb] = np.where(v > 0, np.float32(1.0), np.float32(-1.0))
        else:
            out[:, n, cb] = v * np.float32(2.0) - np.float32(1.0)
    return out.reshape(N_TOT, CO, OH, OW)


_STATE = {}


def _get_nc():
    if "nc" not in _STATE:
        import concourse.bacc as bacc

        nc = bacc.Bacc(
            "TRN2", target_bir_lowering=False, debug=False, num_devices=N_CORES
        )
        _STATE["nc"] = build(nc)
    return _STATE["nc"]


def kernel(x, weight, bias, _trace=False):
    from concourse.bass_utils import run_bass_kernel_spmd

    nc = _get_nc()
    in_maps = prep_inputs(
        np.asarray(x, np.float32),
        np.asarray(weight, np.float32),
        np.asarray(bias, np.float32),
    )
    res = run_bass_kernel_spmd(
        nc, in_maps, core_ids=list(range(N_CORES)), trace=_trace
    )
    _STATE["last_result"] = res
    return gather(res.results)


# revision 33
# speedup vs baseline: 1.0508x; 1.0508x over previous
"""Binarized 3x3 conv + batchnorm(train) + sign, on 8 TRN2 NeuronCores.

Math: out = sign((y - mean)/sqrt(var+eps)) where y = conv(x, sign(w)) + sign(b)
and mean/var are per-channel batch stats.  Since sqrt(var+eps) > 0, the output
is exactly sign(y - mean_c): variance never needs to be computed.  The +-1
channel bias cancels in sign(y - mean), so it is dropped entirely.

Strategy (data-parallel over batch, 4 images/core):
 - implicit GEMM, all matmuls in fp8-e4m3 DoubleRow perf mode (0.5 PE
   cycles/output-row, 2x the fp16 rate; both 128-ci blocks folded into one
   matmul via the DR pair dim).
 - fp32-quality precision from a 3-component split:
       x ~= c1 + c2/64 + c3/64',  c1 = e4m3(x), c2 = e4m3(64*(x-c1)),
       c3 = e4m3(64*(x - c1 - c2/64))
   The /64 scales are folded into the conv weights: comp-1 weights are +-1,
   comp-2/3 weights are +-2^-6 (exact in e4m3, and +-1-weight products are
   exact sign flips).  All 27 matmuls per output tile (3 comps x 9 taps)
   accumulate into ONE fp32 PSUM bank -> a single drain per tile.
   Measured on the reference inputs: 138/23.9M sign flips (rel err 4.8e-3).
 - per-tile drain on ScalarE copies PSUM->SBUF and harvests per-channel sums
   for free (accum_out), leaving VectorE clear for the binarize pass.
 - one tiny AllReduce (128x2 fp32) across the 8 cores for the global mean.
 - pass 2: binarize split over VectorE (is_ge -> {0,1}) and ScalarE
   (Sign -> {-1,0,1}) per BIN_ENG, DMA out as 1 byte/elem; host maps each
   block back to +-1 fp32.
"""

import sys

if "/opt/trn_rl_repo" not in sys.path:
    sys.path.insert(0, "/opt/trn_rl_repo")

import numpy as np
import ml_dtypes

N_CORES = 8
N_PER_CORE = 4          # images per core
CI = 256                # in channels
CO = 256                # out channels
H = W = 56
OH = OW = 54
HWF = H * W             # 3136
HWPAD = HWF + 16        # fp8 image length in SBUF; pair-dim stride 16B-aligned
NPIX = OH * OW          # 2916
RT = 6                  # row tiles per image (9 rows each)
RROWS = 9
FREE = RROWS * W        # 504 raw row span
TFREE = RROWS * OW      # 486 valid outputs per tile
N_TOT = N_CORES * N_PER_CORE
MEAN_SCALE = 1.0 / (N_TOT * NPIX)
C_SCALE = 64.0          # residual components stored at 64x, weights at 1/64
NT = N_PER_CORE * 2 * RT  # 48 tiles per core
# images are DMA'd as head/tail halves with the overlap duplicated so the
# matmul deps are whole-tile (subtile tracking doesn't see rearranged views):
# head serves row tiles 0-2 (reads pix <= 1626), tail serves 3-5 (>= 1512)
HEAD_P = 1632                  # head pix, 16B-aligned pair stride
TAIL_0 = 3 * RROWS * W         # 1512: tail start pix
TAIL_V = HWPAD - TAIL_0        # 1640 valid tail pix (src incl. orig pad)
TAIL_P = 1648                  # padded tail len, 16B-aligned pair stride

# engine per phase-3 binarize block (cb*4+n): v=DVE is_ge {0,1},
# a=ScalarE Sign {-1,0,1}; all stored as fp8 bytes
BIN_ENG = ["v", "a", "v", "a", "v", "a", "v", "v"]

FP8 = ml_dtypes.float8_e4m3


def build(nc, n_cores=N_CORES):
    """Emit the SPMD program into a bacc.Bacc instance."""
    import concourse.mybir as mybir
    from concourse import tile

    f32 = mybir.dt.float32
    fp8 = mybir.dt.float8e4
    ACT = mybir.ActivationFunctionType
    DR = mybir.MatmulPerfMode.DoubleRow

    x_d = [
        nc.dram_tensor(f"x{c}", [N_PER_CORE, 128, 2, HWPAD], fp8, kind="ExternalInput")
        for c in range(3)
    ]
    w1_d = nc.dram_tensor("w1", [128, 2, 9, 2, 128], fp8, kind="ExternalInput")
    ws_d = nc.dram_tensor("ws", [128, 2, 9, 2, 128], fp8, kind="ExternalInput")
    y_d = nc.dram_tensor("y", [N_PER_CORE, 2, 128, NPIX], mybir.dt.uint8, kind="ExternalOutput")

    with tile.TileContext(nc) as tc:
        with (
            tc.tile_pool(name="wpool", bufs=1) as wpool,
            tc.tile_pool(name="xpool", bufs=2) as xpool,
            tc.tile_pool(name="ypool", bufs=1) as ypool,
            tc.tile_pool(name="spool", bufs=1) as spool,
            tc.tile_pool(name="opool", bufs=8) as opool,
            tc.tile_pool(name="pspool", bufs=8, space="PSUM") as pspool,
            tc.tile_pool(name="drampool", bufs=2, space="DRAM") as drampool,
        ):
            w1_sb = wpool.tile([128, 2, 9, 2, 128], fp8, tag="w1")
            ws_sb = wpool.tile([128, 2, 9, 2, 128], fp8, tag="ws")
            y_sb = ypool.tile([128, NT * TFREE], f32)
            sums = spool.tile([128, NT], f32, tag="sums")

            # ---------------- phase 1: conv + drain (+sums) ------------------
            # All DMA transfers serialize on the HWDGE device, so the startup
            # transfers are ordered by first use: w1[cb0] + head of x0 (tile
            # 0's comp-1 matmuls), then ws[cb0]+x1 head, x2 head, tails, cb1
            # weights.
            for n in range(N_PER_CORE):
                xh = [
                    xpool.tile([128, 2, HEAD_P], fp8, tag=f"xh{c}", name=f"xh{c}")
                    for c in range(3)
                ]
                xt = [
                    xpool.tile([128, 2, TAIL_P], fp8, tag=f"xt{c}", name=f"xt{c}")
                    for c in range(3)
                ]
                if n == 0:
                    nc.sync.dma_start(w1_sb[:, 0], w1_d[:, 0])
                    nc.scalar.dma_start(xh[0][:], x_d[0][n][:, :, 0:HEAD_P])
                    nc.sync.dma_start(ws_sb[:, 0], ws_d[:, 0])
                    nc.scalar.dma_start(xh[1][:], x_d[1][n][:, :, 0:HEAD_P])
                    nc.scalar.dma_start(xh[2][:], x_d[2][n][:, :, 0:HEAD_P])
                    nc.sync.dma_start(xt[0][:, :, 0:TAIL_V], x_d[0][n][:, :, TAIL_0:])
                    nc.sync.dma_start(xt[1][:, :, 0:TAIL_V], x_d[1][n][:, :, TAIL_0:])
                    nc.scalar.dma_start(xt[2][:, :, 0:TAIL_V], x_d[2][n][:, :, TAIL_0:])
                    nc.sync.dma_start(w1_sb[:, 1], w1_d[:, 1])
                    nc.scalar.dma_start(ws_sb[:, 1], ws_d[:, 1])
                else:
                    nc.scalar.dma_start(xh[0][:], x_d[0][n][:, :, 0:HEAD_P])
                    nc.scalar.dma_start(xt[0][:, :, 0:TAIL_V], x_d[0][n][:, :, TAIL_0:])
                    nc.gpsimd.dma_start(xh[1][:], x_d[1][n][:, :, 0:HEAD_P])
                    nc.gpsimd.dma_start(xt[1][:, :, 0:TAIL_V], x_d[1][n][:, :, TAIL_0:])
                    nc.gpsimd.dma_start(xh[2][:], x_d[2][n][:, :, 0:HEAD_P])
                    nc.gpsimd.dma_start(xt[2][:, :, 0:TAIL_V], x_d[2][n][:, :, TAIL_0:])

                def emit_mm(ps_t, cb, rt, c, s):
                    w_sb = w1_sb if c == 0 else ws_sb
                    xsrc = xh[c] if rt < 3 else xt[c]
                    base = 0 if rt < 3 else TAIL_0
                    kh, kw = divmod(s, 3)
                    off = (rt * RROWS + kh) * W + kw - base
                    # 4D rhs view drops the 2 wrap cols per row: 486-wide
                    # DR output (0.5 cyc/row on 486 instead of 504)
                    rhs = xsrc[:, :, off : off + FREE].rearrange(
                        "p b (r c) -> p b r c", c=W
                    )[:, :, :, 0:OW]
                    nc.tensor.matmul(
                        ps_t[:],
                        w_sb[:, cb, s],
                        rhs,
                        start=(c == 0 and s == 0),
                        stop=(c == 2 and s == 8),
                        perf_mode=DR,
                    )

                def emit_drain(ps_t, cb, rt):
                    t = (cb * N_PER_CORE + n) * RT + rt
                    nc.scalar.activation(
                        y_sb[:, t * TFREE : (t + 1) * TFREE],
                        ps_t[:],
                        ACT.Copy,
                        accum_out=sums[:, t : t + 1],
                    )

                if n == 0:
                    # first 3 tiles run component-major so the opening 27
                    # matmuls depend only on w1[cb0]+x0 head (the x1/x2
                    # heads stream in behind them on the serial HWDGE)
                    pss = [
                        pspool.tile([128, TFREE], f32, tag="ps", name=f"ps{i}")
                        for i in range(3)
                    ]
                    for c in range(3):
                        for rt in range(3):
                            for s in range(9):
                                emit_mm(pss[rt], 0, rt, c, s)
                    for rt in range(3):
                        emit_drain(pss[rt], 0, rt)
                    rest = [(0, rt) for rt in range(3, RT)] + [
                        (1, rt) for rt in range(RT)
                    ]
                else:
                    rest = [(cb, rt) for cb in range(2) for rt in range(RT)]

                for cb, rt in rest:
                    ps = pspool.tile([128, TFREE], f32, tag="ps")
                    for c in range(3):
                        for s in range(9):
                            emit_mm(ps, cb, rt, c, s)
                    emit_drain(ps, cb, rt)

            # ---------------- phase 2: global mean via AllReduce ------------
            sums2 = spool.tile([128, 2], f32, tag="sums2")
            # tile index t = (cb*N + n)*RT + rt, so cb is outermost:
            # one X-axis reduce over the 24 per-cb columns.
            nc.vector.tensor_reduce(
                sums2[:],
                sums[:].rearrange("p (c m) -> p c m", c=2),
                axis=mybir.AxisListType.X,
                op=mybir.AluOpType.add,
            )
            neg_mean = spool.tile([128, 2], f32, tag="negmean")
            if n_cores > 1:
                cc_in = drampool.tile([128, 2], f32)
                cc_out = drampool.tile([128, 2], f32)
                nc.sync.dma_start(cc_in[:], sums2[:])
                nc.gpsimd.collective_compute(
                    "AllReduce",
                    mybir.AluOpType.add,
                    replica_groups=[list(range(n_cores))],
                    ins=[cc_in.opt()],
                    outs=[cc_out.opt()],
                )
                sums_g = spool.tile([128, 2], f32, tag="sumsg")
                nc.sync.dma_start(sums_g[:], cc_out[:])
                nc.scalar.mul(neg_mean[:], sums_g[:], -MEAN_SCALE)
            else:
                # single-core timing variant (TimelineSim can't model
                # collectives): mean is just this core's sums; on DVE so the
                # reduce -> mul -> first binarize chain stays on one engine
                nc.vector.tensor_scalar(
                    neg_mean[:],
                    sums2[:],
                    -MEAN_SCALE,
                    0.0,
                    mybir.AluOpType.mult,
                    mybir.AluOpType.add,
                )

            # ---------------- phase 3: binarize + store ---------------------
            # spread the 8 blocks over DVE (is_ge -> {0,1}), ScalarE
            # (Sign -> {-1,0,1}) and GpSimd (is_ge) per BIN_ENG; all 1B fp8
            # out.  Host maps back to +-1 fp32 per block encoding.
            # out-DMAs: SP issues blocks 0-4 inline; ACT issues 5-7 after
            # its Signs so issue costs overlap across queues
            bins = []
            for cb in range(2):
                for n in range(N_PER_CORE):
                    b = cb * N_PER_CORE + n
                    t0 = b * RT
                    eng = BIN_ENG[b]
                    bin_t = opool.tile([128, RT * TFREE], fp8, tag="bin")
                    y_v = y_sb[:, t0 * TFREE : (t0 + RT) * TFREE]
                    nm = neg_mean[:, cb : cb + 1]
                    if eng == "a":
                        nc.scalar.activation(bin_t[:], y_v, ACT.Sign, bias=nm)
                    else:
                        nc.vector.tensor_scalar(
                            bin_t[:],
                            y_v,
                            nm,
                            0.0,
                            mybir.AluOpType.add,
                            mybir.AluOpType.is_ge,
                        )
                    bins.append((n, cb, bin_t))
                    if b < 5:
                        nc.sync.dma_start(
                            y_d[n, cb], bin_t[:].bitcast(mybir.dt.uint8)
                        )
            # late blocks' DMAs issue from ACT after its Signs, overlapping
            # SP's issue chain
            for n, cb, bin_t in bins[5:]:
                nc.scalar.dma_start(y_d[n, cb], bin_t[:].bitcast(mybir.dt.uint8))

    nc.compile()
    return nc


def prep_inputs(x, weight, bias):
    """Host-side shard + layout prep. Returns list of 8 per-core input maps."""
    assert x.shape == (N_TOT, CI, H, W) and x.dtype == np.float32

    # x -> [core, n, ci_f(p), ci_b, hw]; 3-component e4m3 split
    xs = np.ascontiguousarray(
        x.reshape(N_CORES, N_PER_CORE, 2, 128, HWF).transpose(0, 1, 3, 2, 4)
    )
    c1 = xs.astype(FP8)
    r1 = xs - c1.astype(np.float32)
    c2 = (r1 * np.float32(C_SCALE)).astype(FP8)
    r2 = r1 - c2.astype(np.float32) * np.float32(1.0 / C_SCALE)
    c3 = (r2 * np.float32(C_SCALE)).astype(FP8)
    pad = ((0, 0),) * 4 + ((0, HWPAD - HWF),)
    c1 = np.pad(c1, pad)
    c2 = np.pad(c2, pad)
    c3 = np.pad(c3, pad)

    wb = np.where(weight >= 0, np.float32(1.0), np.float32(-1.0))
    # [co_b, co_f, ci_b, ci_f, kh, kw] -> [ci_f(p), co_b, (kh kw), ci_b, co_f]
    w6 = wb.reshape(2, 128, 2, 128, 3, 3)
    wt = np.ascontiguousarray(w6.transpose(3, 0, 4, 5, 2, 1)).reshape(
        128, 2, 9, 2, 128
    )
    w1 = wt.astype(FP8)
    ws = (wt * np.float32(1.0 / C_SCALE)).astype(FP8)  # +-2^-6, exact
    return [
        {
            "x0": c1[c],
            "x1": c2[c],
            "x2": c3[c],
            "w1": w1,
            "ws": ws,
        }
        for c in range(N_CORES)
    ]


def gather(results):
    """[{y: [4,2,128,2916] fp8}] * 8 -> (32, 256, 54, 54) fp32 +-1.

    DVE/GpSimd blocks hold {0,1} (is_ge), ScalarE blocks hold {-1,0,1}
    (Sign); see BIN_ENG."""
    ys = np.stack([np.asarray(r["y"]).view(FP8) for r in results]).astype(np.float32)
    out = np.empty_like(ys)
    for b, eng in enumerate(BIN_ENG):
        cb, n = divmod(b, N_PER_CORE)
        v = ys[:, n, cb]
        if eng == "a":
            out[:, n, cb] = np.where(v > 0, np.float32(1.0), np.float32(-1.0))
        else:
            out[:, n, cb] = v * np.float32(2.0) - np.float32(1.0)
    return out.reshape(N_TOT, CO, OH, OW)


_STATE = {}


def _get_nc():
    if "nc" not in _STATE:
        import concourse.bacc as bacc

        nc = bacc.Bacc(
            "TRN2", target_bir_lowering=False, debug=False, num_devices=N_CORES
        )
        _STATE["nc"] = build(nc)
    return _STATE["nc"]


def kernel(x, weight, bias, _trace=False):
    from concourse.bass_utils import run_bass_kernel_spmd

    nc = _get_nc()
    in_maps = prep_inputs(
        np.asarray(x, np.float32),
        np.asarray(weight, np.float32),
        np.asarray(bias, np.float32),
    )
    res = run_bass_kernel_spmd(
        nc, in_maps, core_ids=list(range(N_CORES)), trace=_trace
    )
    _STATE["last_result"] = res
    return gather(res.results)


# revision 40
# speedup vs baseline: 1.0716x; 1.0197x over previous
"""Binarized 3x3 conv + batchnorm(train) + sign, on 8 TRN2 NeuronCores.

Math: out = sign((y - mean)/sqrt(var+eps)) where y = conv(x, sign(w)) + sign(b)
and mean/var are per-channel batch stats.  Since sqrt(var+eps) > 0, the output
is exactly sign(y - mean_c): variance never needs to be computed.  The +-1
channel bias cancels in sign(y - mean), so it is dropped entirely.

Strategy (data-parallel over batch, 4 images/core):
 - implicit GEMM, all matmuls in fp8-e4m3 DoubleRow perf mode (0.5 PE
   cycles/output-row, 2x the fp16 rate; both 128-ci blocks folded into one
   matmul via the DR pair dim).
 - fp32-quality precision from a 3-component split:
       x ~= c1 + c2/64 + c3/64',  c1 = e4m3(x), c2 = e4m3(64*(x-c1)),
       c3 = e4m3(64*(x - c1 - c2/64))
   The /64 scales are folded into the conv weights: comp-1 weights are +-1,
   comp-2/3 weights are +-2^-6 (exact in e4m3, and +-1-weight products are
   exact sign flips).  All 27 matmuls per output tile (3 comps x 9 taps)
   accumulate into ONE fp32 PSUM bank -> a single drain per tile.
   Measured on the reference inputs: 138/23.9M sign flips (rel err 4.8e-3).
 - per-tile drain on ScalarE copies PSUM->SBUF and harvests per-channel sums
   for free (accum_out), leaving VectorE clear for the binarize pass.
 - one tiny AllReduce (128x2 fp32) across the 8 cores for the global mean.
 - pass 2: binarize split over VectorE (is_ge -> {0,1}) and ScalarE
   (Sign -> {-1,0,1}) per BIN_ENG, DMA out as 1 byte/elem; host maps each
   block back to +-1 fp32.
"""

import sys

if "/opt/trn_rl_repo" not in sys.path:
    sys.path.insert(0, "/opt/trn_rl_repo")

import numpy as np
import ml_dtypes

N_CORES = 8
N_PER_CORE = 4          # images per core
CI = 256                # in channels
CO = 256                # out channels
H = W = 56
OH = OW = 54
HWF = H * W             # 3136
HWPAD = HWF + 16        # fp8 image length in SBUF; pair-dim stride 16B-aligned
NPIX = OH * OW          # 2916
RT = 6                  # row tiles per image (9 rows each)
RROWS = 9
FREE = RROWS * W        # 504 raw row span
TFREE = RROWS * OW      # 486 valid outputs per tile
N_TOT = N_CORES * N_PER_CORE
MEAN_SCALE = 1.0 / (N_TOT * NPIX)
C_SCALE = 64.0          # residual components stored at 64x, weights at 1/64
NT = N_PER_CORE * 2 * RT  # 48 tiles per core
# images are DMA'd as head/tail halves with the overlap duplicated so the
# matmul deps are whole-tile (subtile tracking doesn't see rearranged views):
# head serves row tiles 0-2 (reads pix <= 1626), tail serves 3-5 (>= 1512)
HEAD_P = 1632                  # head pix, 16B-aligned pair stride
TAIL_0 = 3 * RROWS * W         # 1512: tail start pix
TAIL_V = HWPAD - TAIL_0        # 1640 valid tail pix (src incl. orig pad)
TAIL_P = 1648                  # padded tail len, 16B-aligned pair stride

# engine per phase-3 binarize block (cb*4+n): v=DVE is_ge {0,1},
# a=ScalarE Sign {-1,0,1}; all stored as fp8 bytes
BIN_ENG = ["v", "a", "v", "a", "v", "a", "v", "v"]

FP8 = ml_dtypes.float8_e4m3


def build(nc, n_cores=N_CORES):
    """Emit the SPMD program into a bacc.Bacc instance."""
    import concourse.mybir as mybir
    from concourse import tile

    f32 = mybir.dt.float32
    fp8 = mybir.dt.float8e4
    ACT = mybir.ActivationFunctionType
    DR = mybir.MatmulPerfMode.DoubleRow

    x_d = [
        nc.dram_tensor(f"x{c}", [N_PER_CORE, 128, 2, HWPAD], fp8, kind="ExternalInput")
        for c in range(3)
    ]
    w1_d = nc.dram_tensor("w1", [128, 2, 9, 2, 128], fp8, kind="ExternalInput")
    ws_d = nc.dram_tensor("ws", [128, 2, 9, 2, 128], fp8, kind="ExternalInput")
    w16_d = nc.dram_tensor("w16", [128, 2, 9, 2, 128], mybir.dt.float16, kind="ExternalInput")
    y_d = nc.dram_tensor("y", [N_PER_CORE, 2, 128, NPIX], mybir.dt.uint8, kind="ExternalOutput")

    with tile.TileContext(nc) as tc:
        with (
            tc.tile_pool(name="wpool", bufs=1) as wpool,
            tc.tile_pool(name="xpool", bufs=2) as xpool,
            tc.tile_pool(name="ypool", bufs=1) as ypool,
            tc.tile_pool(name="spool", bufs=1) as spool,
            tc.tile_pool(name="opool", bufs=8) as opool,
            tc.tile_pool(name="pspool", bufs=6, space="PSUM") as pspool,
            tc.tile_pool(name="pmpool", bufs=2, space="PSUM") as pmpool,
            tc.tile_pool(name="mpool", bufs=1) as mpool,
            tc.tile_pool(name="rpool", bufs=2) as rpool,
            tc.tile_pool(name="drampool", bufs=2, space="DRAM") as drampool,
        ):
            f16 = mybir.dt.float16
            w1_sb = wpool.tile([128, 2, 9, 2, 128], fp8, tag="w1")
            ws_sb = wpool.tile([128, 2, 9, 2, 128], fp8, tag="ws")
            w16_sb = wpool.tile([128, 2, 9, 2, 128], f16, tag="w16")
            nc.sync.dma_start(w16_sb[:], w16_d[:])
            y_sb = ypool.tile([128, NT * TFREE], f32)
            sums = spool.tile([128, NT], f32, tag="sums")

            # ---------------- phase 1: conv + drain (+sums) ------------------
            # All DMA transfers serialize on the HWDGE device, so the startup
            # transfers are ordered by first use: w1[cb0] + head of x0 (tile
            # 0's comp-1 matmuls), then ws[cb0]+x1 head, x2 head, tails, cb1
            # weights.
            for n in range(N_PER_CORE):
                xh = [
                    xpool.tile([128, 2, HEAD_P], fp8, tag=f"xh{c}", name=f"xh{c}")
                    for c in range(3)
                ]
                xt = [
                    xpool.tile([128, 2, TAIL_P], fp8, tag=f"xt{c}", name=f"xt{c}")
                    for c in range(3)
                ]
                if n == 0:
                    nc.sync.dma_start(w1_sb[:, 0], w1_d[:, 0])
                    nc.scalar.dma_start(xh[0][:], x_d[0][n][:, :, 0:HEAD_P])
                    nc.sync.dma_start(ws_sb[:, 0], ws_d[:, 0])
                    nc.scalar.dma_start(xh[1][:], x_d[1][n][:, :, 0:HEAD_P])
                    nc.scalar.dma_start(xh[2][:], x_d[2][n][:, :, 0:HEAD_P])
                    nc.sync.dma_start(xt[0][:, :, 0:TAIL_V], x_d[0][n][:, :, TAIL_0:])
                    nc.sync.dma_start(xt[1][:, :, 0:TAIL_V], x_d[1][n][:, :, TAIL_0:])
                    nc.scalar.dma_start(xt[2][:, :, 0:TAIL_V], x_d[2][n][:, :, TAIL_0:])
                    nc.sync.dma_start(w1_sb[:, 1], w1_d[:, 1])
                    nc.scalar.dma_start(ws_sb[:, 1], ws_d[:, 1])
                else:
                    nc.scalar.dma_start(xh[0][:], x_d[0][n][:, :, 0:HEAD_P])
                    nc.scalar.dma_start(xt[0][:, :, 0:TAIL_V], x_d[0][n][:, :, TAIL_0:])
                    nc.gpsimd.dma_start(xh[1][:], x_d[1][n][:, :, 0:HEAD_P])
                    nc.gpsimd.dma_start(xt[1][:, :, 0:TAIL_V], x_d[1][n][:, :, TAIL_0:])
                    nc.gpsimd.dma_start(xh[2][:], x_d[2][n][:, :, 0:HEAD_P])
                    nc.gpsimd.dma_start(xt[2][:, :, 0:TAIL_V], x_d[2][n][:, :, TAIL_0:])

                # --- last image's windowed input sums on DVE (its part of
                # the mean; images 0-2 use the drain accumulators instead):
                # S[ci, kh*3+kw] = sum over batch+output-window of x shifted
                # by (kh,kw), from row sums R and edge-column corrections.
                AX, ADD = mybir.AxisListType.X, mybir.AluOpType.add
                sn3 = []
                for c in range(3) if n == N_PER_CORE - 1 else ():
                    hv = xh[c][:, :, 0 : 27 * W].rearrange(
                        "p b (h w) -> p b h w", w=W
                    )
                    tv = xt[c][:, :, 0 : 29 * W].rearrange(
                        "p b (h w) -> p b h w", w=W
                    )
                    R = rpool.tile([128, 2, 56], f32, tag=f"R{c}", name=f"R{c}")
                    nc.vector.tensor_reduce(R[:, :, 0:27], hv, axis=AX, op=ADD)
                    nc.vector.tensor_reduce(R[:, :, 27:56], tv, axis=AX, op=ADD)
                    E = rpool.tile([128, 2, 4, 56], f32, tag=f"E{c}", name=f"E{c}")
                    for j, wc in enumerate((0, 1, 54, 55)):
                        nc.vector.tensor_copy(E[:, :, j, 0:27], hv[:, :, :, wc])
                        nc.vector.tensor_copy(E[:, :, j, 27:56], tv[:, :, :, wc])
                    B = rpool.tile([128, 2, 3, 56], f32, tag=f"B{c}", name=f"B{c}")
                    for kw, (ja, jb) in enumerate(((2, 3), (0, 3), (0, 1))):
                        nc.vector.tensor_sub(B[:, :, kw], R[:], E[:, :, ja])
                        nc.vector.tensor_sub(B[:, :, kw], B[:, :, kw], E[:, :, jb])
                    T = rpool.tile([128, 2, 3], f32, tag=f"T{c}", name=f"T{c}")
                    nc.vector.tensor_reduce(T[:], B[:], axis=AX, op=ADD)
                    Sn = rpool.tile([128, 2, 9], f32, tag=f"Sn{c}", name=f"Sn{c}")
                    for kh, (ea, eb) in enumerate(((54, 55), (0, 55), (0, 1))):
                        for kw in range(3):
                            si = kh * 3 + kw
                            nc.vector.tensor_sub(
                                Sn[:, :, si : si + 1],
                                T[:, :, kw : kw + 1],
                                B[:, :, kw, ea : ea + 1],
                            )
                            nc.vector.tensor_sub(
                                Sn[:, :, si : si + 1],
                                Sn[:, :, si : si + 1],
                                B[:, :, kw, eb : eb + 1],
                            )
                    sn3.append(Sn)

                if n == N_PER_CORE - 1:
                    # combine components, cast to fp16, and compute the
                    # per-channel output sums with 36 out-free=1 matmuls
                    # (inserted into the PE stream before image 3's tiles;
                    # their deps are long since ready so no stall)
                    stmp = mpool.tile([128, 2, 9], f32, tag="stmp")
                    s16 = mpool.tile([128, 2, 9], f16, tag="s16")
                    nc.vector.tensor_add(stmp[:], sn3[1][:], sn3[2][:])
                    nc.vector.scalar_tensor_tensor(
                        stmp[:],
                        stmp[:],
                        1.0 / C_SCALE,
                        sn3[0][:],
                        mybir.AluOpType.mult,
                        mybir.AluOpType.add,
                    )
                    nc.vector.tensor_copy(s16[:], stmp[:])
                    pm = [
                        pmpool.tile([128, 1], f32, tag="pm", name=f"pm{cb}")
                        for cb in range(2)
                    ]
                    for cb in range(2):
                        for bb in range(2):
                            for si in range(9):
                                nc.tensor.matmul(
                                    pm[cb][:],
                                    w16_sb[:, cb, si, bb],
                                    s16[:, bb, si : si + 1],
                                    start=(bb == 0 and si == 0),
                                    stop=(bb == 1 and si == 8),
                                )
                    sums2 = spool.tile([128, 2], f32, tag="sums2")
                    for cb in range(2):
                        # 18 drained tiles of images 0-2 for this co block
                        dr = sums[:, cb * 24 : cb * 24 + 18].rearrange(
                            "p (a m) -> p a m", a=1
                        )
                        nc.vector.tensor_reduce(
                            sums2[:, cb : cb + 1], dr, axis=AX, op=ADD
                        )
                        nc.vector.tensor_add(
                            sums2[:, cb : cb + 1], sums2[:, cb : cb + 1], pm[cb][:]
                        )

                def emit_mm(ps_t, cb, rt, c, s):
                    w_sb = w1_sb if c == 0 else ws_sb
                    xsrc = xh[c] if rt < 3 else xt[c]
                    base = 0 if rt < 3 else TAIL_0
                    kh, kw = divmod(s, 3)
                    off = (rt * RROWS + kh) * W + kw - base
                    # 4D rhs view drops the 2 wrap cols per row: 486-wide
                    # DR output (0.5 cyc/row on 486 instead of 504)
                    rhs = xsrc[:, :, off : off + FREE].rearrange(
                        "p b (r c) -> p b r c", c=W
                    )[:, :, :, 0:OW]
                    nc.tensor.matmul(
                        ps_t[:],
                        w_sb[:, cb, s],
                        rhs,
                        start=(c == 0 and s == 0),
                        stop=(c == 2 and s == 8),
                        perf_mode=DR,
                    )

                def emit_drain(ps_t, cb, rt):
                    t = (cb * N_PER_CORE + n) * RT + rt
                    nc.scalar.activation(
                        y_sb[:, t * TFREE : (t + 1) * TFREE],
                        ps_t[:],
                        ACT.Copy,
                        accum_out=sums[:, t : t + 1],
                    )

                if n == 0:
                    # first 3 tiles run component-major so the opening 27
                    # matmuls depend only on w1[cb0]+x0 head (the x1/x2
                    # heads stream in behind them on the serial HWDGE)
                    pss = [
                        pspool.tile([128, TFREE], f32, tag="ps", name=f"ps{i}")
                        for i in range(3)
                    ]
                    for c in range(3):
                        for rt in range(3):
                            for s in range(9):
                                emit_mm(pss[rt], 0, rt, c, s)
                    for rt in range(3):
                        emit_drain(pss[rt], 0, rt)
                    rest = [(0, rt) for rt in range(3, RT)] + [
                        (1, rt) for rt in range(RT)
                    ]
                else:
                    rest = [(cb, rt) for cb in range(2) for rt in range(RT)]

                for cb, rt in rest:
                    ps = pspool.tile([128, TFREE], f32, tag="ps")
                    for c in range(3):
                        for s in range(9):
                            emit_mm(ps, cb, rt, c, s)
                    emit_drain(ps, cb, rt)

            # ---------------- phase 2: global mean via AllReduce ------------
            # sums2 was produced mid-stream (S-route); only the AR remains.
            neg_mean = spool.tile([128, 2], f32, tag="negmean")
            if n_cores > 1:
                cc_in = drampool.tile([128, 2], f32)
                cc_out = drampool.tile([128, 2], f32)
                nc.sync.dma_start(cc_in[:], sums2[:])
                nc.gpsimd.collective_compute(
                    "AllReduce",
                    mybir.AluOpType.add,
                    replica_groups=[list(range(n_cores))],
                    ins=[cc_in.opt()],
                    outs=[cc_out.opt()],
                )
                sums_g = spool.tile([128, 2], f32, tag="sumsg")
                nc.sync.dma_start(sums_g[:], cc_out[:])
                src_sums = sums_g
            else:
                src_sums = sums2
            # on DVE (ScalarE is busy draining until the last tile)
            nc.vector.tensor_scalar(
                neg_mean[:],
                src_sums[:],
                -MEAN_SCALE,
                0.0,
                mybir.AluOpType.mult,
                mybir.AluOpType.add,
            )

            # ---------------- phase 3: binarize + store ---------------------
            # spread the 8 blocks over DVE (is_ge -> {0,1}), ScalarE
            # (Sign -> {-1,0,1}) and GpSimd (is_ge) per BIN_ENG; all 1B fp8
            # out.  Host maps back to +-1 fp32 per block encoding.
            # ---------------- phase 3: binarize + store ---------------------
            # All on DVE (emitted after the S-ops so the DVE queue reaches
            # them as soon as neg_mean lands, mid phase-1 for blocks 0-6).
            # The last block binarizes per tile so only the final tile's
            # drain -> binarize -> small DMA chain sits past the PE stream.
            for b in range(2 * N_PER_CORE):
                cb, n = divmod(b, N_PER_CORE)
                t0 = b * RT
                nm = neg_mean[:, cb : cb + 1]
                if b < 2 * N_PER_CORE - 1:
                    bin_t = opool.tile([128, RT * TFREE], fp8, tag="bin")
                    nc.vector.tensor_scalar(
                        bin_t[:],
                        y_sb[:, t0 * TFREE : (t0 + RT) * TFREE],
                        nm,
                        0.0,
                        mybir.AluOpType.add,
                        mybir.AluOpType.is_ge,
                    )
                    nc.sync.dma_start(y_d[n, cb], bin_t[:].bitcast(mybir.dt.uint8))
                else:
                    for i in range(RT):
                        t = t0 + i
                        bt = opool.tile([128, TFREE], fp8, tag="bint")
                        nc.vector.tensor_scalar(
                            bt[:],
                            y_sb[:, t * TFREE : (t + 1) * TFREE],
                            nm,
                            0.0,
                            mybir.AluOpType.add,
                            mybir.AluOpType.is_ge,
                        )
                        nc.sync.dma_start(
                            y_d[n, cb][:, i * TFREE : (i + 1) * TFREE],
                            bt[:].bitcast(mybir.dt.uint8),
                        )

    nc.compile()
    return nc


def prep_inputs(x, weight, bias):
    """Host-side shard + layout prep. Returns list of 8 per-core input maps."""
    assert x.shape == (N_TOT, CI, H, W) and x.dtype == np.float32

    # x -> [core, n, ci_f(p), ci_b, hw]; 3-component e4m3 split
    xs = np.ascontiguousarray(
        x.reshape(N_CORES, N_PER_CORE, 2, 128, HWF).transpose(0, 1, 3, 2, 4)
    )
    c1 = xs.astype(FP8)
    r1 = xs - c1.astype(np.float32)
    c2 = (r1 * np.float32(C_SCALE)).astype(FP8)
    r2 = r1 - c2.astype(np.float32) * np.float32(1.0 / C_SCALE)
    c3 = (r2 * np.float32(C_SCALE)).astype(FP8)
    pad = ((0, 0),) * 4 + ((0, HWPAD - HWF),)
    c1 = np.pad(c1, pad)
    c2 = np.pad(c2, pad)
    c3 = np.pad(c3, pad)

    wb = np.where(weight >= 0, np.float32(1.0), np.float32(-1.0))
    # [co_b, co_f, ci_b, ci_f, kh, kw] -> [ci_f(p), co_b, (kh kw), ci_b, co_f]
    w6 = wb.reshape(2, 128, 2, 128, 3, 3)
    wt = np.ascontiguousarray(w6.transpose(3, 0, 4, 5, 2, 1)).reshape(
        128, 2, 9, 2, 128
    )
    w1 = wt.astype(FP8)
    ws = (wt * np.float32(1.0 / C_SCALE)).astype(FP8)  # +-2^-6, exact
    w16 = wt.astype(np.float16)                        # +-1 for the mean GEMV
    return [
        {
            "x0": c1[c],
            "x1": c2[c],
            "x2": c3[c],
            "w1": w1,
            "ws": ws,
            "w16": w16,
        }
        for c in range(N_CORES)
    ]


def gather(results):
    """[{y: [4,2,128,2916] fp8 {0,1}}] * 8 -> (32, 256, 54, 54) fp32 +-1."""
    ys = np.stack([np.asarray(r["y"]).view(FP8) for r in results]).astype(np.float32)
    return ys.reshape(N_TOT, CO, OH, OW) * np.float32(2.0) - np.float32(1.0)


_STATE = {}


def _get_nc():
    if "nc" not in _STATE:
        import concourse.bacc as bacc

        nc = bacc.Bacc(
            "TRN2", target_bir_lowering=False, debug=False, num_devices=N_CORES
        )
        _STATE["nc"] = build(nc)
    return _STATE["nc"]


def kernel(x, weight, bias, _trace=False):
    from concourse.bass_utils import run_bass_kernel_spmd

    nc = _get_nc()
    in_maps = prep_inputs(
        np.asarray(x, np.float32),
        np.asarray(weight, np.float32),
        np.asarray(bias, np.float32),
    )
    res = run_bass_kernel_spmd(
        nc, in_maps, core_ids=list(range(N_CORES)), trace=_trace
    )
    _STATE["last_result"] = res
    return gather(res.results)


# revision 41
# speedup vs baseline: 1.1178x; 1.0431x over previous
"""Binarized 3x3 conv + batchnorm(train) + sign, on 8 TRN2 NeuronCores.

Math: out = sign((y - mean)/sqrt(var+eps)) where y = conv(x, sign(w)) + sign(b)
and mean/var are per-channel batch stats.  Since sqrt(var+eps) > 0, the output
is exactly sign(y - mean_c): variance never needs to be computed.  The +-1
channel bias cancels in sign(y - mean), so it is dropped entirely.

Strategy (data-parallel over batch, 4 images/core):
 - implicit GEMM, all matmuls in fp8-e4m3 DoubleRow perf mode (0.5 PE
   cycles/output-row, 2x the fp16 rate; both 128-ci blocks folded into one
   matmul via the DR pair dim).
 - fp32-quality precision from a 3-component split:
       x ~= c1 + c2/64 + c3/64',  c1 = e4m3(x), c2 = e4m3(64*(x-c1)),
       c3 = e4m3(64*(x - c1 - c2/64))
   The /64 scales are folded into the conv weights: comp-1 weights are +-1,
   comp-2/3 weights are +-2^-6 (exact in e4m3, and +-1-weight products are
   exact sign flips).  All 27 matmuls per output tile (3 comps x 9 taps)
   accumulate into ONE fp32 PSUM bank -> a single drain per tile.
   Measured on the reference inputs: 138/23.9M sign flips (rel err 4.8e-3).
 - per-tile drain on ScalarE copies PSUM->SBUF and harvests per-channel sums
   for free (accum_out), leaving VectorE clear for the binarize pass.
 - one tiny AllReduce (128x2 fp32) across the 8 cores for the global mean.
 - pass 2: binarize split over VectorE (is_ge -> {0,1}) and ScalarE
   (Sign -> {-1,0,1}) per BIN_ENG, DMA out as 1 byte/elem; host maps each
   block back to +-1 fp32.
"""

import sys

if "/opt/trn_rl_repo" not in sys.path:
    sys.path.insert(0, "/opt/trn_rl_repo")

import numpy as np
import ml_dtypes

N_CORES = 8
N_PER_CORE = 4          # images per core
CI = 256                # in channels
CO = 256                # out channels
H = W = 56
OH = OW = 54
HWF = H * W             # 3136
HWPAD = HWF + 16        # fp8 image length in SBUF; pair-dim stride 16B-aligned
NPIX = OH * OW          # 2916
RT = 6                  # row tiles per image (9 rows each)
RROWS = 9
FREE = RROWS * W        # 504 raw row span
TFREE = RROWS * OW      # 486 valid outputs per tile
N_TOT = N_CORES * N_PER_CORE
MEAN_SCALE = 1.0 / (N_TOT * NPIX)
C_SCALE = 64.0          # residual components stored at 64x, weights at 1/64
NT = N_PER_CORE * 2 * RT  # 48 tiles per core
# images are DMA'd as head/tail halves with the overlap duplicated so the
# matmul deps are whole-tile (subtile tracking doesn't see rearranged views):
# head serves row tiles 0-2 (reads pix <= 1626), tail serves 3-5 (>= 1512)
HEAD_P = 1632                  # head pix, 16B-aligned pair stride
TAIL_0 = 3 * RROWS * W         # 1512: tail start pix
TAIL_V = HWPAD - TAIL_0        # 1640 valid tail pix (src incl. orig pad)
TAIL_P = 1648                  # padded tail len, 16B-aligned pair stride

# engine per phase-3 binarize block (cb*4+n): v=DVE is_ge {0,1},
# a=ScalarE Sign {-1,0,1}; all stored as fp8 bytes
BIN_ENG = ["v", "a", "v", "a", "v", "a", "v", "v"]

FP8 = ml_dtypes.float8_e4m3


def build(nc, n_cores=N_CORES):
    """Emit the SPMD program into a bacc.Bacc instance."""
    import concourse.mybir as mybir
    from concourse import tile

    f32 = mybir.dt.float32
    fp8 = mybir.dt.float8e4
    ACT = mybir.ActivationFunctionType
    DR = mybir.MatmulPerfMode.DoubleRow

    x_d = [
        nc.dram_tensor(f"x{c}", [N_PER_CORE, 128, 2, HWPAD], fp8, kind="ExternalInput")
        for c in range(3)
    ]
    w1_d = nc.dram_tensor("w1", [128, 2, 9, 2, 128], fp8, kind="ExternalInput")
    ws_d = nc.dram_tensor("ws", [128, 2, 9, 2, 128], fp8, kind="ExternalInput")
    w16_d = nc.dram_tensor("w16", [128, 2, 9, 2, 128], mybir.dt.float16, kind="ExternalInput")
    y_d = nc.dram_tensor("y", [N_PER_CORE, 2, 128, NPIX], mybir.dt.uint8, kind="ExternalOutput")

    with tile.TileContext(nc) as tc:
        with (
            tc.tile_pool(name="wpool", bufs=1) as wpool,
            tc.tile_pool(name="xpool", bufs=2) as xpool,
            tc.tile_pool(name="ypool", bufs=1) as ypool,
            tc.tile_pool(name="spool", bufs=1) as spool,
            tc.tile_pool(name="opool", bufs=8) as opool,
            tc.tile_pool(name="pspool", bufs=6, space="PSUM") as pspool,
            tc.tile_pool(name="pmpool", bufs=2, space="PSUM") as pmpool,
            tc.tile_pool(name="mpool", bufs=1) as mpool,
            tc.tile_pool(name="rpool", bufs=2) as rpool,
            tc.tile_pool(name="drampool", bufs=2, space="DRAM") as drampool,
        ):
            f16 = mybir.dt.float16
            w1_sb = wpool.tile([128, 2, 9, 2, 128], fp8, tag="w1")
            ws_sb = wpool.tile([128, 2, 9, 2, 128], fp8, tag="ws")
            w16_sb = wpool.tile([128, 2, 9, 2, 128], f16, tag="w16")
            y_sb = ypool.tile([128, NT * TFREE], f32)
            sums = spool.tile([128, NT], f32, tag="sums")

            # ---------------- phase 1: conv + drain (+sums) ------------------
            # All DMA transfers serialize on the HWDGE device, so the startup
            # transfers are ordered by first use: w1[cb0] + head of x0 (tile
            # 0's comp-1 matmuls), then ws[cb0]+x1 head, x2 head, tails, cb1
            # weights.
            for n in range(N_PER_CORE):
                xh = [
                    xpool.tile([128, 2, HEAD_P], fp8, tag=f"xh{c}", name=f"xh{c}")
                    for c in range(3)
                ]
                xt = [
                    xpool.tile([128, 2, TAIL_P], fp8, tag=f"xt{c}", name=f"xt{c}")
                    for c in range(3)
                ]
                if n == 0:
                    nc.sync.dma_start(w1_sb[:, 0], w1_d[:, 0])
                    nc.scalar.dma_start(xh[0][:], x_d[0][n][:, :, 0:HEAD_P])
                    nc.sync.dma_start(ws_sb[:, 0], ws_d[:, 0])
                    nc.scalar.dma_start(xh[1][:], x_d[1][n][:, :, 0:HEAD_P])
                    nc.scalar.dma_start(xh[2][:], x_d[2][n][:, :, 0:HEAD_P])
                    nc.sync.dma_start(xt[0][:, :, 0:TAIL_V], x_d[0][n][:, :, TAIL_0:])
                    nc.sync.dma_start(xt[1][:, :, 0:TAIL_V], x_d[1][n][:, :, TAIL_0:])
                    nc.scalar.dma_start(xt[2][:, :, 0:TAIL_V], x_d[2][n][:, :, TAIL_0:])
                    nc.sync.dma_start(w1_sb[:, 1], w1_d[:, 1])
                    nc.scalar.dma_start(ws_sb[:, 1], ws_d[:, 1])
                    # mean-GEMV weights: not needed until ~100us, keep the
                    # transfer behind the startup-critical ones
                    nc.sync.dma_start(w16_sb[:], w16_d[:])
                else:
                    nc.scalar.dma_start(xh[0][:], x_d[0][n][:, :, 0:HEAD_P])
                    nc.scalar.dma_start(xt[0][:, :, 0:TAIL_V], x_d[0][n][:, :, TAIL_0:])
                    nc.gpsimd.dma_start(xh[1][:], x_d[1][n][:, :, 0:HEAD_P])
                    nc.gpsimd.dma_start(xt[1][:, :, 0:TAIL_V], x_d[1][n][:, :, TAIL_0:])
                    nc.gpsimd.dma_start(xh[2][:], x_d[2][n][:, :, 0:HEAD_P])
                    nc.gpsimd.dma_start(xt[2][:, :, 0:TAIL_V], x_d[2][n][:, :, TAIL_0:])

                # --- last image's windowed input sums on DVE (its part of
                # the mean; images 0-2 use the drain accumulators instead):
                # S[ci, kh*3+kw] = sum over batch+output-window of x shifted
                # by (kh,kw), from row sums R and edge-column corrections.
                AX, ADD = mybir.AxisListType.X, mybir.AluOpType.add
                sn3 = []
                for c in range(3) if n == N_PER_CORE - 1 else ():
                    hv = xh[c][:, :, 0 : 27 * W].rearrange(
                        "p b (h w) -> p b h w", w=W
                    )
                    tv = xt[c][:, :, 0 : 29 * W].rearrange(
                        "p b (h w) -> p b h w", w=W
                    )
                    R = rpool.tile([128, 2, 56], f32, tag=f"R{c}", name=f"R{c}")
                    nc.vector.tensor_reduce(R[:, :, 0:27], hv, axis=AX, op=ADD)
                    nc.vector.tensor_reduce(R[:, :, 27:56], tv, axis=AX, op=ADD)
                    E = rpool.tile([128, 2, 4, 56], f32, tag=f"E{c}", name=f"E{c}")
                    for j, wc in enumerate((0, 1, 54, 55)):
                        nc.vector.tensor_copy(E[:, :, j, 0:27], hv[:, :, :, wc])
                        nc.vector.tensor_copy(E[:, :, j, 27:56], tv[:, :, :, wc])
                    B = rpool.tile([128, 2, 3, 56], f32, tag=f"B{c}", name=f"B{c}")
                    for kw, (ja, jb) in enumerate(((2, 3), (0, 3), (0, 1))):
                        nc.vector.tensor_sub(B[:, :, kw], R[:], E[:, :, ja])
                        nc.vector.tensor_sub(B[:, :, kw], B[:, :, kw], E[:, :, jb])
                    T = rpool.tile([128, 2, 3], f32, tag=f"T{c}", name=f"T{c}")
                    nc.vector.tensor_reduce(T[:], B[:], axis=AX, op=ADD)
                    Sn = rpool.tile([128, 2, 9], f32, tag=f"Sn{c}", name=f"Sn{c}")
                    for kh, (ea, eb) in enumerate(((54, 55), (0, 55), (0, 1))):
                        for kw in range(3):
                            si = kh * 3 + kw
                            nc.vector.tensor_sub(
                                Sn[:, :, si : si + 1],
                                T[:, :, kw : kw + 1],
                                B[:, :, kw, ea : ea + 1],
                            )
                            nc.vector.tensor_sub(
                                Sn[:, :, si : si + 1],
                                Sn[:, :, si : si + 1],
                                B[:, :, kw, eb : eb + 1],
                            )
                    sn3.append(Sn)

                if n == N_PER_CORE - 1:
                    # combine components, cast to fp16, and compute the
                    # per-channel output sums with 36 out-free=1 matmuls
                    # (inserted into the PE stream before image 3's tiles;
                    # their deps are long since ready so no stall)
                    stmp = mpool.tile([128, 2, 9], f32, tag="stmp")
                    s16 = mpool.tile([128, 2, 9], f16, tag="s16")
                    nc.vector.tensor_add(stmp[:], sn3[1][:], sn3[2][:])
                    nc.vector.scalar_tensor_tensor(
                        stmp[:],
                        stmp[:],
                        1.0 / C_SCALE,
                        sn3[0][:],
                        mybir.AluOpType.mult,
                        mybir.AluOpType.add,
                    )
                    nc.vector.tensor_copy(s16[:], stmp[:])
                    pm = [
                        pmpool.tile([128, 1], f32, tag="pm", name=f"pm{cb}")
                        for cb in range(2)
                    ]
                    for cb in range(2):
                        for bb in range(2):
                            for si in range(9):
                                nc.tensor.matmul(
                                    pm[cb][:],
                                    w16_sb[:, cb, si, bb],
                                    s16[:, bb, si : si + 1],
                                    start=(bb == 0 and si == 0),
                                    stop=(bb == 1 and si == 8),
                                )
                    sums2 = spool.tile([128, 2], f32, tag="sums2")
                    for cb in range(2):
                        # 18 drained tiles of images 0-2 for this co block
                        dr = sums[:, cb * 24 : cb * 24 + 18].rearrange(
                            "p (a m) -> p a m", a=1
                        )
                        nc.vector.tensor_reduce(
                            sums2[:, cb : cb + 1], dr, axis=AX, op=ADD
                        )
                        nc.vector.tensor_add(
                            sums2[:, cb : cb + 1], sums2[:, cb : cb + 1], pm[cb][:]
                        )

                def emit_mm(ps_t, cb, rt, c, s):
                    w_sb = w1_sb if c == 0 else ws_sb
                    xsrc = xh[c] if rt < 3 else xt[c]
                    base = 0 if rt < 3 else TAIL_0
                    kh, kw = divmod(s, 3)
                    off = (rt * RROWS + kh) * W + kw - base
                    # 4D rhs view drops the 2 wrap cols per row: 486-wide
                    # DR output (0.5 cyc/row on 486 instead of 504)
                    rhs = xsrc[:, :, off : off + FREE].rearrange(
                        "p b (r c) -> p b r c", c=W
                    )[:, :, :, 0:OW]
                    nc.tensor.matmul(
                        ps_t[:],
                        w_sb[:, cb, s],
                        rhs,
                        start=(c == 0 and s == 0),
                        stop=(c == 2 and s == 8),
                        perf_mode=DR,
                    )

                def emit_drain(ps_t, cb, rt):
                    t = (cb * N_PER_CORE + n) * RT + rt
                    nc.scalar.activation(
                        y_sb[:, t * TFREE : (t + 1) * TFREE],
                        ps_t[:],
                        ACT.Copy,
                        accum_out=sums[:, t : t + 1],
                    )

                if n == 0:
                    # first 3 tiles run component-major so the opening 27
                    # matmuls depend only on w1[cb0]+x0 head (the x1/x2
                    # heads stream in behind them on the serial HWDGE)
                    pss = [
                        pspool.tile([128, TFREE], f32, tag="ps", name=f"ps{i}")
                        for i in range(3)
                    ]
                    for c in range(3):
                        for rt in range(3):
                            for s in range(9):
                                emit_mm(pss[rt], 0, rt, c, s)
                    for rt in range(3):
                        emit_drain(pss[rt], 0, rt)
                    rest = [(0, rt) for rt in range(3, RT)] + [
                        (1, rt) for rt in range(RT)
                    ]
                else:
                    rest = [(cb, rt) for cb in range(2) for rt in range(RT)]

                for cb, rt in rest:
                    ps = pspool.tile([128, TFREE], f32, tag="ps")
                    for c in range(3):
                        for s in range(9):
                            emit_mm(ps, cb, rt, c, s)
                    emit_drain(ps, cb, rt)

            # ---------------- phase 2: global mean via AllReduce ------------
            # sums2 was produced mid-stream (S-route); only the AR remains.
            neg_mean = spool.tile([128, 2], f32, tag="negmean")
            if n_cores > 1:
                cc_in = drampool.tile([128, 2], f32)
                cc_out = drampool.tile([128, 2], f32)
                nc.sync.dma_start(cc_in[:], sums2[:])
                nc.gpsimd.collective_compute(
                    "AllReduce",
                    mybir.AluOpType.add,
                    replica_groups=[list(range(n_cores))],
                    ins=[cc_in.opt()],
                    outs=[cc_out.opt()],
                )
                sums_g = spool.tile([128, 2], f32, tag="sumsg")
                nc.sync.dma_start(sums_g[:], cc_out[:])
                src_sums = sums_g
            else:
                src_sums = sums2
            # on DVE (ScalarE is busy draining until the last tile)
            nc.vector.tensor_scalar(
                neg_mean[:],
                src_sums[:],
                -MEAN_SCALE,
                0.0,
                mybir.AluOpType.mult,
                mybir.AluOpType.add,
            )

            # ---------------- phase 3: binarize + store ---------------------
            # spread the 8 blocks over DVE (is_ge -> {0,1}), ScalarE
            # (Sign -> {-1,0,1}) and GpSimd (is_ge) per BIN_ENG; all 1B fp8
            # out.  Host maps back to +-1 fp32 per block encoding.
            # ---------------- phase 3: binarize + store ---------------------
            # All on DVE (emitted after the S-ops so the DVE queue reaches
            # them as soon as neg_mean lands, mid phase-1 for blocks 0-6).
            # The last block binarizes per tile so only the final tile's
            # drain -> binarize -> small DMA chain sits past the PE stream.
            for b in range(2 * N_PER_CORE):
                cb, n = divmod(b, N_PER_CORE)
                t0 = b * RT
                nm = neg_mean[:, cb : cb + 1]
                if b < 2 * N_PER_CORE - 1:
                    bin_t = opool.tile([128, RT * TFREE], fp8, tag="bin")
                    nc.vector.tensor_scalar(
                        bin_t[:],
                        y_sb[:, t0 * TFREE : (t0 + RT) * TFREE],
                        nm,
                        0.0,
                        mybir.AluOpType.add,
                        mybir.AluOpType.is_ge,
                    )
                    nc.sync.dma_start(y_d[n, cb], bin_t[:].bitcast(mybir.dt.uint8))
                else:
                    for i in range(RT):
                        t = t0 + i
                        bt = opool.tile([128, TFREE], fp8, tag="bint")
                        nc.vector.tensor_scalar(
                            bt[:],
                            y_sb[:, t * TFREE : (t + 1) * TFREE],
                            nm,
                            0.0,
                            mybir.AluOpType.add,
                            mybir.AluOpType.is_ge,
                        )
                        nc.sync.dma_start(
                            y_d[n, cb][:, i * TFREE : (i + 1) * TFREE],
                            bt[:].bitcast(mybir.dt.uint8),
                        )

    nc.compile()
    return nc


def prep_inputs(x, weight, bias):
    """Host-side shard + layout prep. Returns list of 8 per-core input maps."""
    assert x.shape == (N_TOT, CI, H, W) and x.dtype == np.float32

    # x -> [core, n, ci_f(p), ci_b, hw]; 3-component e4m3 split
    xs = np.ascontiguousarray(
        x.reshape(N_CORES, N_PER_CORE, 2, 128, HWF).transpose(0, 1, 3, 2, 4)
    )
    c1 = xs.astype(FP8)
    r1 = xs - c1.astype(np.float32)
    c2 = (r1 * np.float32(C_SCALE)).astype(FP8)
    r2 = r1 - c2.astype(np.float32) * np.float32(1.0 / C_SCALE)
    c3 = (r2 * np.float32(C_SCALE)).astype(FP8)
    pad = ((0, 0),) * 4 + ((0, HWPAD - HWF),)
    c1 = np.pad(c1, pad)
    c2 = np.pad(c2, pad)
    c3 = np.pad(c3, pad)

    wb = np.where(weight >= 0, np.float32(1.0), np.float32(-1.0))
    # [co_b, co_f, ci_b, ci_f, kh, kw] -> [ci_f(p), co_b, (kh kw), ci_b, co_f]
    w6 = wb.reshape(2, 128, 2, 128, 3, 3)
    wt = np.ascontiguousarray(w6.transpose(3, 0, 4, 5, 2, 1)).reshape(
        128, 2, 9, 2, 128
    )
    w1 = wt.astype(FP8)
    ws = (wt * np.float32(1.0 / C_SCALE)).astype(FP8)  # +-2^-6, exact
    w16 = wt.astype(np.float16)                        # +-1 for the mean GEMV
    return [
        {
            "x0": c1[c],
            "x1": c2[c],
            "x2": c3[c],
            "w1": w1,
            "ws": ws,
            "w16": w16,
        }
        for c in range(N_CORES)
    ]


def gather(results):
    """[{y: [4,2,128,2916] fp8 {0,1}}] * 8 -> (32, 256, 54, 54) fp32 +-1."""
    ys = np.stack([np.asarray(r["y"]).view(FP8) for r in results]).astype(np.float32)
    return ys.reshape(N_TOT, CO, OH, OW) * np.float32(2.0) - np.float32(1.0)


_STATE = {}


def _get_nc():
    if "nc" not in _STATE:
        import concourse.bacc as bacc

        nc = bacc.Bacc(
            "TRN2", target_bir_lowering=False, debug=False, num_devices=N_CORES
        )
        _STATE["nc"] = build(nc)
    return _STATE["nc"]


def kernel(x, weight, bias, _trace=False):
    from concourse.bass_utils import run_bass_kernel_spmd

    nc = _get_nc()
    in_maps = prep_inputs(
        np.asarray(x, np.float32),
        np.asarray(weight, np.float32),
        np.asarray(bias, np.float32),
    )
    res = run_bass_kernel_spmd(
        nc, in_maps, core_ids=list(range(N_CORES)), trace=_trace
    )
    _STATE["last_result"] = res
    return gather(res.results)


# revision 45
# speedup vs baseline: 1.2437x; 1.1127x over previous
"""Binarized 3x3 conv + batchnorm(train) + sign, on 8 TRN2 NeuronCores.

Math: out = sign((y - mean)/sqrt(var+eps)) where y = conv(x, sign(w)) + sign(b)
and mean/var are per-channel batch stats.  Since sqrt(var+eps) > 0, the output
is exactly sign(y - mean_c): variance never needs to be computed.  The +-1
channel bias cancels in sign(y - mean), so it is dropped entirely.

Strategy (data-parallel over batch, 4 images/core):
 - 1-D Winograd F(2,3) along W (host-side input transform in fp32 during
   prep): 1.5x fewer MACs than direct conv.  d~[j] = B^T d per output-column
   pair (27 pairs), kernel g~ = G g in {+-1, +-1/2, +-3/2} (exact in e4m3),
   vertical taps stay direct (3 kh).  y_even = m0+m1+m2, y_odd = m1-m2-m3.
 - all matmuls fp8-e4m3 DoubleRow (0.5 PE cycles/output-row), 243-wide
   contiguous rhs slices (no wrap waste), 36 matmuls per output tile
   (4 j x 3 comps x 3 kh) into 4 PSUM accumulators.
 - fp32-quality via the 3-component split d~ ~= c1 + c2/64 + c3/64 with the
   /64 folded into the weights (g~, g~/64).  Measured on the reference
   inputs: 148/23.9M sign flips (rel err 5.0e-3).
 - drains combine the 4 j-accumulators into y_sb even|odd halves on
   VectorE/GpSimd, harvesting per-channel sums for the mean for free.
 - one tiny AllReduce (128x2 fp32) across the 8 cores for the global mean.
 - binarize (y + (-mean)) >= 0 on VectorE/GpSimd as fp8 {0,1} bytes, the
   last block per-tile so only one small chain trails the PE stream; host
   de-interleaves even/odd and maps to +-1 fp32.
"""

import sys

if "/opt/trn_rl_repo" not in sys.path:
    sys.path.insert(0, "/opt/trn_rl_repo")

import numpy as np
import ml_dtypes

N_CORES = 8
N_PER_CORE = 4          # images per core
CI = 256                # in channels
CO = 256                # out channels
H = W = 56
OH = OW = 54
NPIX = OH * OW          # 2916
RT = 6                  # row tiles per image (9 rows each)
RROWS = 9
NP_ = 27                # output column pairs
JP = 4                  # Winograd positions
TFREE = RROWS * NP_     # 243 outputs per tile half (even or odd)
N_TOT = N_CORES * N_PER_CORE
MEAN_SCALE = 1.0 / (N_TOT * NPIX)
C_SCALE = 64.0          # residual components stored at 64x, weights at 1/64
NT = N_PER_CORE * 2 * RT  # 48 tiles per core
# head rows 0-28, tail rows 27-55 (29 rows each); per-j plane padded to 784
# so the ci-block pair stride (4*784=3136B) stays 16B-aligned
HROWS = 29
TAIL_R0 = 27
JPAD = HROWS * NP_ + 1  # 784

FP8 = ml_dtypes.float8_e4m3


def build(nc, n_cores=N_CORES):
    """Emit the SPMD program into a bacc.Bacc instance."""
    import concourse.mybir as mybir
    from concourse import tile

    f32 = mybir.dt.float32
    fp8 = mybir.dt.float8e4
    DR = mybir.MatmulPerfMode.DoubleRow
    ADD = mybir.AluOpType.add
    MUL = mybir.AluOpType.mult

    xh_d = [
        nc.dram_tensor(f"xh{c}", [N_PER_CORE, 128, 2, JP, JPAD], fp8, kind="ExternalInput")
        for c in range(3)
    ]
    xt_d = [
        nc.dram_tensor(f"xt{c}", [N_PER_CORE, 128, 2, JP, JPAD], fp8, kind="ExternalInput")
        for c in range(3)
    ]
    w1_d = nc.dram_tensor("w1", [128, 2, JP, 3, 2, 128], fp8, kind="ExternalInput")
    ws_d = nc.dram_tensor("ws", [128, 2, JP, 3, 2, 128], fp8, kind="ExternalInput")
    y_d = nc.dram_tensor("y", [N_PER_CORE, 2, 128, NPIX], mybir.dt.uint8, kind="ExternalOutput")

    with tile.TileContext(nc) as tc:
        with (
            tc.tile_pool(name="wpool", bufs=1) as wpool,
            tc.tile_pool(name="xpool", bufs=2) as xpool,
            tc.tile_pool(name="ypool", bufs=1) as ypool,
            tc.tile_pool(name="spool", bufs=1) as spool,
            tc.tile_pool(name="opool", bufs=2) as opool,
            tc.tile_pool(name="btpool", bufs=6) as btpool,
            tc.tile_pool(name="pspool", bufs=4, space="PSUM") as pspool,
            tc.tile_pool(name="drampool", bufs=2, space="DRAM") as drampool,
        ):
            w1_sb = wpool.tile([128, 2, JP, 3, 2, 128], fp8, tag="w1")
            ws_sb = wpool.tile([128, 2, JP, 3, 2, 128], fp8, tag="ws")
            y_sb = ypool.tile([128, NT * 2 * TFREE], f32)
            sums = spool.tile([128, 2 * NT], f32, tag="sums")

            # ---------------- phase 1: conv + drain (+sums) ------------------
            # HWDGE serializes all transfers; order startup by first use.
            # First 3-tile group runs component-major so the opening matmuls
            # need only w1[cb0] + comp-1 heads (j01 first).
            for n in range(N_PER_CORE):
                xh = [
                    xpool.tile([128, 2, JP, JPAD], fp8, tag=f"xh{c}", name=f"xh{c}")
                    for c in range(3)
                ]
                xt = [
                    xpool.tile([128, 2, JP, JPAD], fp8, tag=f"xt{c}", name=f"xt{c}")
                    for c in range(3)
                ]
                if n == 0:
                    nc.sync.dma_start(w1_sb[:, 0], w1_d[:, 0])
                    nc.scalar.dma_start(xh[0][:, :, 0:2], xh_d[0][n][:, :, 0:2])
                    nc.sync.dma_start(ws_sb[:, 0], ws_d[:, 0])
                    nc.scalar.dma_start(xh[1][:, :, 0:2], xh_d[1][n][:, :, 0:2])
                    nc.sync.dma_start(xh[0][:, :, 2:4], xh_d[0][n][:, :, 2:4])
                    nc.scalar.dma_start(xh[2][:, :, 0:2], xh_d[2][n][:, :, 0:2])
                    nc.sync.dma_start(xh[1][:, :, 2:4], xh_d[1][n][:, :, 2:4])
                    nc.scalar.dma_start(xh[2][:, :, 2:4], xh_d[2][n][:, :, 2:4])
                    nc.sync.dma_start(xt[0][:], xt_d[0][n])
                    nc.scalar.dma_start(xt[1][:], xt_d[1][n])
                    nc.sync.dma_start(xt[2][:], xt_d[2][n])
                    nc.sync.dma_start(w1_sb[:, 1], w1_d[:, 1])
                    nc.scalar.dma_start(ws_sb[:, 1], ws_d[:, 1])
                else:
                    nc.scalar.dma_start(xh[0][:], xh_d[0][n])
                    nc.scalar.dma_start(xt[0][:], xt_d[0][n])
                    nc.scalar.dma_start(xh[1][:], xh_d[1][n])
                    nc.scalar.dma_start(xt[1][:], xt_d[1][n])
                    nc.scalar.dma_start(xh[2][:], xh_d[2][n])
                    nc.scalar.dma_start(xt[2][:], xt_d[2][n])

                def emit_mm(ps_t, cb, rt, j, c, kh):
                    w_sb = w1_sb if c == 0 else ws_sb
                    row = rt * RROWS + kh
                    if rt < 3:
                        src, base = xh[c], 0
                    else:
                        src, base = xt[c], TAIL_R0
                    off = (row - base) * NP_
                    nc.tensor.matmul(
                        ps_t[:, j, 0:TFREE],
                        w_sb[:, cb, j, kh],
                        src[:, :, j, off : off + TFREE],
                        # psum groups are bank-granular: j0+j1 share a bank
                        # (one group), j2+j3 the other
                        start=(c == 0 and kh == 0 and j in (0, 2)),
                        stop=(c == 2 and kh == 2 and j in (1, 3)),
                        perf_mode=DR,
                    )

                def emit_drain(ps_t, cb, rt):
                    t = (cb * N_PER_CORE + n) * RT + rt
                    ev = y_sb[:, 2 * t * TFREE : (2 * t + 1) * TFREE]
                    od = y_sb[:, (2 * t + 1) * TFREE : (2 * t + 2) * TFREE]
                    p0 = ps_t[:, 0, 0:TFREE]
                    p1 = ps_t[:, 1, 0:TFREE]
                    p2 = ps_t[:, 2, 0:TFREE]
                    p3 = ps_t[:, 3, 0:TFREE]
                    # even = m0+m1+m2 on GpSimd, odd = m1-m2-m3 on DVE
                    nc.gpsimd.scalar_tensor_tensor(ev, p0, 1.0, p1, MUL, ADD)
                    nc.gpsimd.scalar_tensor_tensor(
                        ev, p2, 1.0, ev, MUL, ADD,
                        accum_out=sums[:, 2 * t : 2 * t + 1],
                    )
                    nc.vector.scalar_tensor_tensor(od, p2, -1.0, p1, MUL, ADD)
                    nc.vector.scalar_tensor_tensor(
                        od, p3, -1.0, od, MUL, ADD,
                        accum_out=sums[:, 2 * t + 1 : 2 * t + 2],
                    )

                if n == 0:
                    pss = [
                        pspool.tile([128, JP, 256], f32, tag="ps", name=f"ps{i}")
                        for i in range(3)
                    ]
                    for c in range(3):
                        for j in range(JP):
                            for kh in range(3):
                                for i in range(3):
                                    emit_mm(pss[i], 0, i, j, c, kh)
                    for i in range(3):
                        emit_drain(pss[i], 0, i)
                    rest = [(0, rt) for rt in range(3, RT)] + [
                        (1, rt) for rt in range(RT)
                    ]
                else:
                    rest = [(cb, rt) for cb in range(2) for rt in range(RT)]

                for cb, rt in rest:
                    ps = pspool.tile([128, JP, 256], f32, tag="ps")
                    for j in range(JP):
                        for c in range(3):
                            for kh in range(3):
                                emit_mm(ps, cb, rt, j, c, kh)
                    emit_drain(ps, cb, rt)

            # ---------------- phase 2: global mean via AllReduce ------------
            sums2 = spool.tile([128, 2], f32, tag="sums2")
            nc.vector.tensor_reduce(
                sums2[:],
                sums[:].rearrange("p (c m) -> p c m", c=2),
                axis=mybir.AxisListType.X,
                op=ADD,
            )
            neg_mean = spool.tile([128, 2], f32, tag="negmean")
            if n_cores > 1:
                cc_in = drampool.tile([128, 2], f32)
                cc_out = drampool.tile([128, 2], f32)
                nc.sync.dma_start(cc_in[:], sums2[:])
                nc.gpsimd.collective_compute(
                    "AllReduce",
                    ADD,
                    replica_groups=[list(range(n_cores))],
                    ins=[cc_in.opt()],
                    outs=[cc_out.opt()],
                )
                sums_g = spool.tile([128, 2], f32, tag="sumsg")
                nc.sync.dma_start(sums_g[:], cc_out[:])
                src_sums = sums_g
            else:
                src_sums = sums2
            nc.vector.tensor_scalar(
                neg_mean[:], src_sums[:], -MEAN_SCALE, 0.0, MUL, ADD
            )

            # ---------------- phase 3: binarize + store ---------------------
            # DVE takes 6 blocks (incl. the last, per-tile), GpSimd 2.
            IS_GE = mybir.AluOpType.is_ge
            for b in range(2 * N_PER_CORE):
                cb, n = divmod(b, N_PER_CORE)
                t0 = b * RT
                nm = neg_mean[:, cb : cb + 1]
                if b == 2 * N_PER_CORE - 1:
                    for i in range(RT):
                        t = t0 + i
                        bt = btpool.tile([128, 2 * TFREE], fp8, tag="bint")
                        nc.vector.tensor_scalar(
                            bt[:],
                            y_sb[:, 2 * t * TFREE : (2 * t + 2) * TFREE],
                            nm, 0.0, ADD, IS_GE,
                        )
                        nc.sync.dma_start(
                            y_d[n, cb][:, i * 2 * TFREE : (i + 1) * 2 * TFREE],
                            bt[:].bitcast(mybir.dt.uint8),
                        )
                else:
                    e = nc.gpsimd if b in (1, 4) else nc.vector
                    bin_t = opool.tile([128, RT * 2 * TFREE], fp8, tag="bin")
                    e.tensor_scalar(
                        bin_t[:],
                        y_sb[:, 2 * t0 * TFREE : 2 * (t0 + RT) * TFREE],
                        nm, 0.0, ADD, IS_GE,
                    )
                    nc.sync.dma_start(y_d[n, cb], bin_t[:].bitcast(mybir.dt.uint8))

    nc.compile()
    return nc


def prep_inputs(x, weight, bias):
    """Host-side shard + Winograd transform + fp8 split."""
    assert x.shape == (N_TOT, CI, H, W) and x.dtype == np.float32

    xs = np.ascontiguousarray(
        x.reshape(N_CORES, N_PER_CORE, 2, 128, H, W).transpose(0, 1, 3, 2, 4, 5)
    )  # [core, n, ci_f, ci_b, 56, 56]
    a = xs[..., 0:54:2]
    b = xs[..., 1:55:2]
    c = xs[..., 2:56:2]
    d = xs[..., 3:56:2]
    dt = np.stack([a - c, b + c, c - b, b - d], axis=4)  # [.., ci_b, j, 56h, 27]

    c1 = dt.astype(FP8)
    r1 = dt - c1.astype(np.float32)
    c2 = (r1 * np.float32(C_SCALE)).astype(FP8)
    r2 = r1 - c2.astype(np.float32) * np.float32(1.0 / C_SCALE)
    c3 = (r2 * np.float32(C_SCALE)).astype(FP8)

    def halves(cq):
        # [core, n, 128, 2, 4, 56, 27] -> head rows 0-28, tail rows 27-55
        hd = cq[..., 0:HROWS, :].reshape(N_CORES, N_PER_CORE, 128, 2, JP, HROWS * NP_)
        tl = cq[..., TAIL_R0:, :].reshape(N_CORES, N_PER_CORE, 128, 2, JP, HROWS * NP_)
        pad = ((0, 0),) * 5 + ((0, JPAD - HROWS * NP_),)
        return np.pad(hd, pad), np.pad(tl, pad)

    hs, ts = zip(*(halves(q) for q in (c1, c2, c3)))

    wb = np.where(weight >= 0, np.float32(1.0), np.float32(-1.0))
    g0 = wb[:, :, :, 0]
    g1 = wb[:, :, :, 1]
    g2 = wb[:, :, :, 2]
    gt = np.stack(
        [g0, (g0 + g1 + g2) / 2, (g0 - g1 + g2) / 2, g2], axis=3
    ).astype(np.float32)  # [co, ci, kh, j]
    # [co_b, co_f, ci_b, ci_f, kh, j] -> [ci_f, co_b, j, kh, ci_b, co_f]
    g6 = gt.reshape(2, 128, 2, 128, 3, JP)
    wt = np.ascontiguousarray(g6.transpose(3, 0, 5, 4, 2, 1))
    w1 = wt.astype(FP8)
    ws = (wt * np.float32(1.0 / C_SCALE)).astype(FP8)
    assert np.all(w1.astype(np.float32) == wt)
    assert np.all(ws.astype(np.float32) * C_SCALE == wt)

    out = []
    for core in range(N_CORES):
        m = {"w1": w1, "ws": ws}
        for ci in range(3):
            m[f"xh{ci}"] = hs[ci][core]
            m[f"xt{ci}"] = ts[ci][core]
        out.append(m)
    return out


def gather(results):
    """[{y: [4,2,128,2916] fp8 {0,1}}] * 8 -> (32, 256, 54, 54) fp32 +-1.

    Per row-tile the 486 bytes are [even 9x27 | odd 9x27]; de-interleave."""
    ys = np.stack([np.asarray(r["y"]).view(FP8) for r in results]).astype(np.float32)
    ys = ys.reshape(N_CORES, N_PER_CORE, 2, 128, RT, 2, RROWS, NP_)
    out = np.empty((N_CORES, N_PER_CORE, 2, 128, RT, RROWS, OW), np.float32)
    out[..., 0::2] = ys[:, :, :, :, :, 0]
    out[..., 1::2] = ys[:, :, :, :, :, 1]
    return out.reshape(N_TOT, CO, OH, OW) * np.float32(2.0) - np.float32(1.0)


_STATE = {}


def _get_nc():
    if "nc" not in _STATE:
        import concourse.bacc as bacc

        nc = bacc.Bacc(
            "TRN2", target_bir_lowering=False, debug=False, num_devices=N_CORES
        )
        _STATE["nc"] = build(nc)
    return _STATE["nc"]


def kernel(x, weight, bias, _trace=False):
    from concourse.bass_utils import run_bass_kernel_spmd

    nc = _get_nc()
    in_maps = prep_inputs(
        np.asarray(x, np.float32),
        np.asarray(weight, np.float32),
        np.asarray(bias, np.float32),
    )
    res = run_bass_kernel_spmd(
        nc, in_maps, core_ids=list(range(N_CORES)), trace=_trace
    )
    _STATE["last_result"] = res
    return gather(res.results)
